# revision 1
# baseline (speedup 1.0000x reference)
"""Trainium2 Bass kernel for nn_CrossCompressUnit.

Reference computation (B rows, D=64):
    s_vv[b] = e[b] . w_vv      s_ev[b] = v[b] . w_ev
    s_ve[b] = e[b] . w_ve      s_ee[b] = v[b] . w_ee
    v_out[b] = v[b]*s_vv[b] + e[b]*s_ev[b] + bias_v
    e_out[b] = v[b]*s_ve[b] + e[b]*s_ee[b] + bias_e

Strategy (pure data-parallel over 8 cores, 32768 rows/core):
  rows are mapped 2-per-partition (row = base + 2p + u) so every DMA moves
  512B contiguous per partition.  Per 256-row "unit":
    - PE transposes the [128,128] v/e tiles into PSUM (identity matmul)
    - ScalarE copies the transposes PSUM->SBUF in 512-wide chunks
    - PE computes all four dot products as one matmul per tensor against a
      [128,8] block-diagonal weight matrix ([W;0] | [0;W])
    - DVE does the four per-row scaled copies (tensor_scalar, fp32 2x mode)
    - PE sums the pairs via identity-matmul PSUM accumulation
    - ScalarE copies outputs PSUM->SBUF, DMA stores them
"""

import os
import numpy as np

D = 64
N_CORES = 8
P = 128
ROWS_FULL = 262144

last_exec_time_ns = None
last_results = None

_BUILD_CACHE = {}


def _split_sync_waits(nc):
    """Walrus in this container rejects >1 sync wait per engine instruction
    (setupSyncWait: "Too many sync wait commands").  Tile emits multi-wait
    instructions freely, so split the extras onto sequencer NOPs inserted
    just before, each carrying one wait."""
    import concourse.mybir as mybir

    isa = nc.isa
    acc = {
        mybir.EngineType.DVE: nc.vector,
        mybir.EngineType.PE: nc.tensor,
        mybir.EngineType.Pool: nc.gpsimd,
        mybir.EngineType.Activation: nc.scalar,
        mybir.EngineType.SP: nc.sync,
    }
    n = 0
    for f in nc.m.functions:
        for b in f.blocks:
            new_list = []
            for i in b.instructions:
                si = i.sync_info
                if (
                    si is not None
                    and si.on_wait
                    and len(si.on_wait) > 1
                    and i.engine in acc
                ):
                    waits = list(si.on_wait)
                    for w in waits[:-1]:
                        nop = acc[i.engine]._isa(
                            isa.Opcode.NEURON_ISA_TPB_OPCODE_NOP, {}, None, [], [], True
                        )
                        nop.sync_info = mybir.SyncInfo(on_wait=[w], on_update=[])
                        new_list.append(nop)
                        n += 1
                    i.sync_info = mybir.SyncInfo(
                        on_wait=[waits[-1]], on_update=list(si.on_update or [])
                    )
                new_list.append(i)
            b.instructions[:] = new_list
    return n


def _build_bass(rows: int, with_bias: bool, units_per_group: int = 4,
                split_waits: bool = True, reps: int = 1,
                psum_scalars: bool = False, dma_batch: int = 1,
                ps_cfg=(1, 2, 2), dma_only: bool = False,
                compute_only: bool = False, big_tt: bool = True):
    from contextlib import ExitStack

    import concourse.bass as bass
    import concourse.mybir as mybir
    from concourse import tile
    from concourse.masks import make_identity

    f32 = mybir.dt.float32
    U = units_per_group
    group_rows = U * 2 * P
    assert rows % group_rows == 0, (rows, group_rows)
    n_groups = rows // group_rows

    nc = bass.Bass("TRN2", target_bir_lowering=False, debug=False)

    v_d = nc.dram_tensor("v", [rows, D], f32, kind="ExternalInput").ap()
    e_d = nc.dram_tensor("e", [rows, D], f32, kind="ExternalInput").ap()
    # Rows: w_vv, w_ve, w_ev, w_ee — stacked host-side so one DMA loads all.
    w4_d = nc.dram_tensor("w4", [4, D], f32, kind="ExternalInput").ap()
    # Rows: bias_v, bias_e.
    bias_d = nc.dram_tensor("bias2", [2, D], f32, kind="ExternalInput").ap()
    vout_d = nc.dram_tensor("v_out", [rows, D], f32, kind="ExternalOutput").ap()
    eout_d = nc.dram_tensor("e_out", [rows, D], f32, kind="ExternalOutput").ap()

    # row = ((g*U + j)*P + p)*2 + u   ->  [g][p, j, u, d]
    v_r = v_d.rearrange("(g j p u) d -> g p j u d", j=U, p=P, u=2)
    e_r = e_d.rearrange("(g j p u) d -> g p j u d", j=U, p=P, u=2)
    vout_r = vout_d.rearrange("(g j p u) d -> g p j u d", j=U, p=P, u=2)
    eout_r = eout_d.rearrange("(g j p u) d -> g p j u d", j=U, p=P, u=2)
    if dma_batch == 2:
        assert n_groups % 2 == 0
        v_r2 = v_d.rearrange("(h k j p u) d -> h p k j u d", k=2, j=U, p=P, u=2)
        e_r2 = e_d.rearrange("(h k j p u) d -> h p k j u d", k=2, j=U, p=P, u=2)

    with tile.TileContext(nc) as tc, ExitStack() as ctx:
        consts = ctx.enter_context(tc.tile_pool(name="consts", bufs=1))
        inp = ctx.enter_context(tc.tile_pool(name="inp", bufs=3))
        tT = ctx.enter_context(tc.tile_pool(name="tT", bufs=2))
        dsb = ctx.enter_context(tc.tile_pool(name="dsb", bufs=3))
        tmix = ctx.enter_context(tc.tile_pool(name="tmix", bufs=6))
        osb = ctx.enter_context(tc.tile_pool(name="osb", bufs=3))
        psT = ctx.enter_context(tc.tile_pool(name="psT", bufs=ps_cfg[0], space="PSUM"))
        psD = ctx.enter_context(tc.tile_pool(name="psD", bufs=ps_cfg[1], space="PSUM"))
        psO = ctx.enter_context(tc.tile_pool(name="psO", bufs=ps_cfg[2], space="PSUM"))

        ident = consts.tile([P, P], f32)
        make_identity(nc, ident[:])

        # Stack the four weight vectors as rows, transpose on PE, then build
        # W_AB[128,8]: cols 0:4 = [W;0] (even-slot rows), cols 4:8 = [0;W].
        w4 = consts.tile([4, D], f32)
        nc.sync.dma_start(out=w4[:], in_=w4_d[:])
        wT_ps = psD.tile([P, U * 16], f32, tag="d_ps")
        nc.tensor.transpose(wT_ps[0:D, 0:4], w4[:, :], ident[0:4, 0:4])
        w_ab = consts.tile([P, 8], f32)
        nc.gpsimd.memset(w_ab[:], 0.0)
        nc.vector.tensor_copy(out=w_ab[0:D, 0:4], in_=wT_ps[0:D, 0:4])
        nc.vector.tensor_copy(out=w_ab[D:P, 4:8], in_=wT_ps[0:D, 0:4])

        if with_bias:
            # rowsel.T @ biasrow broadcasts biasrow's row 0 to all partitions.
            rowsel = consts.tile([P, P], f32)
            nc.gpsimd.memset(rowsel[:], 0.0)
            nc.gpsimd.memset(rowsel[0:1, :], 1.0)
            biasrow_v = consts.tile([P, 2 * D], f32)
            biasrow_e = consts.tile([P, 2 * D], f32)
            nc.gpsimd.memset(biasrow_v[:], 0.0)
            nc.gpsimd.memset(biasrow_e[:], 0.0)
            nc.sync.dma_start(out=biasrow_v[0:1, 0:D], in_=bias_d[0:1, :])
            nc.sync.dma_start(out=biasrow_v[0:1, D : 2 * D], in_=bias_d[0:1, :])
            nc.sync.dma_start(out=biasrow_e[0:1, 0:D], in_=bias_d[1:2, :])
            nc.sync.dma_start(out=biasrow_e[0:1, D : 2 * D], in_=bias_d[1:2, :])

        v_sb2 = e_sb2 = None
        for g in [g for _ in range(reps) for g in range(n_groups)]:
            if dma_batch == 2:
                if g % 2 == 0:
                    v_sb2 = inp.tile([P, 2, U, 2, D], f32, tag="v_sb")
                    e_sb2 = inp.tile([P, 2, U, 2, D], f32, tag="e_sb")
                    nc.sync.dma_start(out=v_sb2[:], in_=v_r2[g // 2])
                    nc.sync.dma_start(out=e_sb2[:], in_=e_r2[g // 2])
                v_sb = v_sb2[:, g % 2]
                e_sb = e_sb2[:, g % 2]
            else:
                v_sb = inp.tile([P, U, 2, D], f32, tag="v_sb")
                e_sb = inp.tile([P, U, 2, D], f32, tag="e_sb")
                if compute_only:
                    nc.gpsimd.memset(v_sb[:], 1.0)
                    nc.gpsimd.memset(e_sb[:], 1.0)
                else:
                    nc.sync.dma_start(out=v_sb[:], in_=v_r[g])
                    nc.sync.dma_start(out=e_sb[:], in_=e_r[g])

            if dma_only:
                vo_sb = osb.tile([P, U, 2, D], f32, tag="vo_sb")
                eo_sb = osb.tile([P, U, 2, D], f32, tag="eo_sb")
                nc.vector.tensor_copy(out=vo_sb[:, 0, 0, 0:1], in_=v_sb[:, 0, 0, 0:1])
                nc.vector.tensor_copy(out=eo_sb[:, 0, 0, 0:1], in_=e_sb[:, 0, 0, 0:1])
                nc.sync.dma_start(out=vout_r[g], in_=vo_sb[:])
                nc.sync.dma_start(out=eout_r[g], in_=eo_sb[:])
                continue

            vT_ps = psT.tile([P, U * P], f32, tag="vT_ps")
            eT_ps = psT.tile([P, U * P], f32, tag="eT_ps")
            for j in range(U):
                nc.tensor.transpose(vT_ps[:, j * P : (j + 1) * P], v_sb[:, j], ident[:])
                nc.tensor.transpose(eT_ps[:, j * P : (j + 1) * P], e_sb[:, j], ident[:])
            vT_sb = tT.tile([P, U * P], f32, tag="vT_sb")
            eT_sb = tT.tile([P, U * P], f32, tag="eT_sb")
            nc.scalar.copy(out=vT_sb[:], in_=vT_ps[:])
            nc.scalar.copy(out=eT_sb[:], in_=eT_ps[:])

            # d_ps cols per unit j: [v dots: A 0:4, B 4:8 | e dots: +8]
            d_ps = psD.tile([P, U * 16], f32, tag="d_ps")
            for j in range(U):
                b = j * 16
                nc.tensor.matmul(
                    d_ps[:, b : b + 8], vT_sb[:, j * P : (j + 1) * P], w_ab[:]
                )
                nc.tensor.matmul(
                    d_ps[:, b + 8 : b + 16], eT_sb[:, j * P : (j + 1) * P], w_ab[:]
                )
            if psum_scalars:
                d_sb = d_ps
            else:
                d_sb = dsb.tile([P, U * 16], f32, tag="d_sb")
                nc.vector.tensor_copy(out=d_sb[:], in_=d_ps[:])

            o_ps = psO.tile([P, 2 * U * P], f32, tag="o_ps")
            if big_tt and not with_bias:
                # One tensor_tensor per (input, output-role) for the whole
                # group: scalar broadcast along d via a step-0 AP into d_sb.
                # d_sb cols per unit j: j*16 + [vA(0:4) vB(4:8) eA(8:12) eB(12:16)],
                # within each 4: [w_vv, w_ve, w_ev, w_ee].
                t1 = tmix.tile([P, U, 2, D], f32, tag="t1")
                t2 = tmix.tile([P, U, 2, D], f32, tag="t2")
                t3 = tmix.tile([P, U, 2, D], f32, tag="t3")
                t4 = tmix.tile([P, U, 2, D], f32, tag="t4")

                def bc(col0):
                    return bass.AP(
                        tensor=d_sb.tensor,
                        offset=d_sb[:, col0 : col0 + 1].offset,
                        ap=[d_sb.ap[0], [16, U], [4, 2], [0, D]],
                    )

                mult = mybir.AluOpType.mult
                nc.vector.tensor_tensor(out=t1[:], in0=v_sb[:], in1=bc(8), op=mult)
                nc.vector.tensor_tensor(out=t2[:], in0=e_sb[:], in1=bc(2), op=mult)
                nc.vector.tensor_tensor(out=t3[:], in0=v_sb[:], in1=bc(9), op=mult)
                nc.vector.tensor_tensor(out=t4[:], in0=e_sb[:], in1=bc(3), op=mult)
                half = U * P
                nc.tensor.matmul(o_ps[:, 0:half], ident[:], t1[:], start=True, stop=False)
                nc.tensor.matmul(o_ps[:, 0:half], ident[:], t2[:], start=False, stop=True)
                nc.tensor.matmul(o_ps[:, half:], ident[:], t3[:], start=True, stop=False)
                nc.tensor.matmul(o_ps[:, half:], ident[:], t4[:], start=False, stop=True)
            else:
                for j in range(U):
                    t1 = tmix.tile([P, 2, D], f32, tag="t1")
                    t2 = tmix.tile([P, 2, D], f32, tag="t2")
                    t3 = tmix.tile([P, 2, D], f32, tag="t3")
                    t4 = tmix.tile([P, 2, D], f32, tag="t4")
                    for u in range(2):
                        cv = j * 16 + u * 4
                        ce = cv + 8
                        nc.vector.tensor_scalar_mul(
                            t1[:, u], v_sb[:, j, u], d_sb[:, ce + 0 : ce + 1]
                        )
                        nc.vector.tensor_scalar_mul(
                            t2[:, u], e_sb[:, j, u], d_sb[:, cv + 2 : cv + 3]
                        )
                        nc.vector.tensor_scalar_mul(
                            t3[:, u], v_sb[:, j, u], d_sb[:, ce + 1 : ce + 2]
                        )
                        nc.vector.tensor_scalar_mul(
                            t4[:, u], e_sb[:, j, u], d_sb[:, cv + 3 : cv + 4]
                        )
                    vsl = slice(j * P, (j + 1) * P)
                    esl = slice(U * P + j * P, U * P + (j + 1) * P)
                    nc.tensor.matmul(o_ps[:, vsl], ident[:], t1[:], start=True, stop=False)
                    nc.tensor.matmul(
                        o_ps[:, vsl], ident[:], t2[:], start=False, stop=not with_bias
                    )
                    nc.tensor.matmul(o_ps[:, esl], ident[:], t3[:], start=True, stop=False)
                    nc.tensor.matmul(
                        o_ps[:, esl], ident[:], t4[:], start=False, stop=not with_bias
                    )
                    if with_bias:
                        nc.tensor.matmul(
                            o_ps[:, vsl], rowsel[:], biasrow_v[:], start=False, stop=True
                        )
                        nc.tensor.matmul(
                            o_ps[:, esl], rowsel[:], biasrow_e[:], start=False, stop=True
                        )

            vo_sb = osb.tile([P, U, 2, D], f32, tag="vo_sb")
            eo_sb = osb.tile([P, U, 2, D], f32, tag="eo_sb")
            nc.scalar.copy(out=vo_sb[:], in_=o_ps[:, 0 : U * P])
            nc.scalar.copy(out=eo_sb[:], in_=o_ps[:, U * P : 2 * U * P])
            if not compute_only:
                # Output DMAs go out on the second HWDGE ring (ACT) so their
                # per-DMA fixed costs overlap with the input ring (SP).
                nc.scalar.dma_start(out=vout_r[g], in_=vo_sb[:])
                nc.scalar.dma_start(out=eout_r[g], in_=eo_sb[:])

    if split_waits:
        _split_sync_waits(nc)
    return nc


def _get_bass(rows: int, with_bias: bool):
    key = (rows, with_bias)
    if key not in _BUILD_CACHE:
        _BUILD_CACHE[key] = _build_bass(rows, with_bias)
    return _BUILD_CACHE[key]


def kernel(v, e, w_vv, w_ev, w_ve, w_ee, bias_v, bias_e):
    global last_exec_time_ns, last_results
    from concourse.bass_utils import run_bass_kernel_spmd

    v = np.ascontiguousarray(np.asarray(v, dtype=np.float32))
    e = np.ascontiguousarray(np.asarray(e, dtype=np.float32))
    rows = v.shape[0]
    assert rows % N_CORES == 0
    shard = rows // N_CORES

    with_bias = bool(np.any(np.asarray(bias_v)) or np.any(np.asarray(bias_e)))
    nc = _get_bass(shard, with_bias)

    consts = {
        "w4": np.stack(
            [
                np.asarray(w_vv, np.float32).reshape(D),
                np.asarray(w_ve, np.float32).reshape(D),
                np.asarray(w_ev, np.float32).reshape(D),
                np.asarray(w_ee, np.float32).reshape(D),
            ]
        ),
        "bias2": np.stack(
            [
                np.asarray(bias_v, np.float32).reshape(D),
                np.asarray(bias_e, np.float32).reshape(D),
            ]
        ),
    }
    in_maps = []
    for i in range(N_CORES):
        m = dict(consts)
        m["v"] = v[i * shard : (i + 1) * shard]
        m["e"] = e[i * shard : (i + 1) * shard]
        in_maps.append(m)

    trace = os.environ.get("BASS_KERNEL_TRACE", "0") == "1"
    try:
        res = run_bass_kernel_spmd(
            nc, in_maps, core_ids=list(range(N_CORES)), trace=trace
        )
    except ModuleNotFoundError:
        # NTFF profiling hook not available in this container.
        res = run_bass_kernel_spmd(
            nc, in_maps, core_ids=list(range(N_CORES)), trace=False
        )
    last_exec_time_ns = res.exec_time_ns
    last_results = res

    v_out = np.concatenate([res.results[i]["v_out"] for i in range(N_CORES)], axis=0)
    e_out = np.concatenate([res.results[i]["e_out"] for i in range(N_CORES)], axis=0)
    return (v_out, e_out)

